# revision 16
# baseline (speedup 1.0000x reference)
"""Trainium2 Bass kernel for PixelSNAIL-style strict-causal attention.

Problem: query/key/value [B=4, H=64, W=64, C=256] fp32.
  S = 4096 tokens per batch; scores = (Q K^T)/16 with strict causal mask
  (position i attends to j < i); out = softmax(scores) @ V (row 0 -> 0).

Strategy (8 NeuronCores, v3):
  - 2 cores per batch, context-parallel over k: core h owns rows
    64h..64h+63 of EVERY 128-row k-block (row-half split). Both cores run
    the IDENTICAL program (SPMD) and their partial numerator/denominator
    outputs are summed on host. The row-half split (vs block parity) makes
    the diagonal-block mask pattern identical across slots AND cores, so a
    single [128, 512] additive-mask input covers everything and the
    per-slot trimming is h-independent.
  - All matmul operands bf16 (fp32 PSUM accumulation). Measured L2 rel
    err ~3e-3 (gate 2e-2). PE streams at ~2.0 GHz under the P0 power
    limit, so the matmul cycle count (~140k/core) is the real floor.
  - No max-subtraction in softmax (scores ~ N(0,1), exp safe in fp32).
  - V gets a ones-column (row 257) so the softmax denominator accumulates
    in PSUM alongside the numerator.
  - q-slots (512 rows) processed ASCENDING as one globally software-
    pipelined stream of k-position PAIRS: St(g+2) is issued before PV(g),
    so each pair's exp (ACT) hides under other pairs' matmuls and the PE
    never drains (keeps the HAM clock gate open). Slot-local order puts
    the diagonal (masked) pair first; PSUM->SBUF output copies drain
    qs2/qs3 first because the next slot's first PV MMs target them.
  - St is pair-granular [128, 1024] (2 PSUM banks) with ONE activation
    per pair: the ~460ns fixed ACT overhead is paid 44x, not 72x.

Layout per core (b = core//2, h = core%2):
  qt_in [256, 4096] bf16 = Q[b]^T
  kt_in [256, 2048] bf16 = row-half-packed K[b]^T (local col 64*blk + r,
                            r in 0..63 <-> global row 128*blk + 64h + r)
  v_in  [2048, 257] bf16 = row-half-packed V[b] ++ ones column
  m_in  [128, 256] fp32  = additive diag mask [E|O]
  o_out [4096, 257] fp32 = partial (numerator ++ denominator)

St pair layouts (matmul PSUM writes must not cross a bank boundary):
  Diagonal pair (slot p, k-positions 2p and 2p+1), width 768:
    cols 0:512   = pos 2p   vs q sub-blocks 0..3 (qs0,1 masked by m)
    cols 512:768 = pos 2p+1 vs q sub-blocks 2,3  (masked by m)
  Non-diag pair (t < p), width 1024:
    cols 0:512 = pos 2t, cols 512:1024 = pos 2t+1, no mask.
"""

import numpy as np
from ml_dtypes import bfloat16

B = 4
S = 4096          # 64*64 tokens per batch
C = 256
NPOS = 16         # 128-row local k positions per core
NSLOT = 8         # q slots of 512 rows
VW = 257          # V width incl. denominator ones-column
SCALE = 1.0 / 16.0
NEG = -1.0e30

_CACHE = {}


def _build_nc():
    import concourse.bacc as bacc
    import concourse.mybir as mybir
    import concourse.tile as tile

    F32 = mybir.dt.float32
    BF16 = mybir.dt.bfloat16
    AluAdd = mybir.AluOpType.add
    Exp = mybir.ActivationFunctionType.Exp

    nc = bacc.Bacc("TRN2", target_bir_lowering=False, debug=False, num_devices=8)
    qt_in = nc.dram_tensor("qt_in", [C, S], BF16, kind="ExternalInput").ap()
    kt_in = nc.dram_tensor("kt_in", [C, NPOS * 128], BF16, kind="ExternalInput").ap()
    v_in = nc.dram_tensor("v_in", [NPOS * 128, VW], BF16, kind="ExternalInput").ap()
    m_in = nc.dram_tensor("m_in", [128, 256], F32, kind="ExternalInput").ap()
    o_out = nc.dram_tensor("o_out", [S, VW], F32, kind="ExternalOutput").ap()

    with tile.TileContext(nc) as tc:
        with (
            tc.tile_pool(name="const", bufs=1) as const,
            tc.tile_pool(name="pt", bufs=3) as ptp,
            tc.tile_pool(name="osb", bufs=3) as osbp,
            tc.tile_pool(name="st", bufs=2, space="PSUM") as stp,
            tc.tile_pool(name="op", bufs=4, space="PSUM") as opp,
        ):
            # PE warmup: tiny matmuls issued during the DMA preamble so the
            # HAM clock gate opens before real work.
            wu = const.tile([128, 64], BF16, tag="wu")
            nc.gpsimd.memset(wu[:], 0.0)
            wu_ps = stp.tile([128, 1024], F32, tag="st", name="wu_ps")
            for _ in range(48):
                nc.tensor.matmul(wu_ps[0:64, 0:64], lhsT=wu[:], rhs=wu[:],
                                 start=True, stop=True)

            qt = [const.tile([128, S], BF16, tag=f"qt{c}", name=f"qt{c}")
                  for c in range(2)]
            kt = [const.tile([128, NPOS * 128], BF16, tag=f"kt{c}", name=f"kt{c}")
                  for c in range(2)]
            vsb = const.tile([128, NPOS * VW], BF16, tag="v")
            mask = const.tile([128, 256], F32, tag="m")

            def kt_dma(eng, c0, c1):
                for c in range(2):
                    eng.dma_start(kt[c][:, c0:c1], kt_in[c * 128:(c + 1) * 128, c0:c1])

            def qt_dma(eng, s0, s1):
                for c in range(2):
                    eng.dma_start(
                        qt[c][:, s0 * 512:s1 * 512],
                        qt_in[c * 128:(c + 1) * 128, s0 * 512:s1 * 512],
                    )

            def v_dma(eng, pos, npos):
                eng.dma_start(
                    vsb[:, pos * VW:(pos + npos) * VW].rearrange(
                        "p (t v) -> p t v", t=npos
                    ),
                    v_in[pos * 128:(pos + npos) * 128, :].rearrange(
                        "(t p) v -> p t v", p=128
                    ),
                )

            # Pre-load the Exp activation table (one-time ~1.3us) on the ACT
            # queue during the preamble, so slot 0's first real exp does not
            # pay it. The probe input must be fp32 like the real exps (the
            # table is input-dtype specific).
            wu32 = const.tile([128, 1], F32, tag="wu32")
            nc.gpsimd.memset(wu32[:], 0.0)
            wu_pt = const.tile([128, 1], BF16, tag="wupt")
            nc.scalar.activation(wu_pt[:], wu32[:], Exp, scale=SCALE)

            # Input DMAs on four parallel queues, strictly in consumption
            # order with small slot-0/1 chunks first (each trigger costs
            # ~600-750ns of queue-sequencer time; aggregate HBM bandwidth
            # makes slot-1 delivery the tight spot). qt c-halves split over
            # scalar+vector; kt halves over sync+gpsimd; v+mask on gpsimd;
            # outputs appended to the sync queue later.
            def kt1_dma(eng, c, c0, c1):
                eng.dma_start(kt[c][:, c0:c1], kt_in[c * 128:(c + 1) * 128, c0:c1])

            def qt1_dma(eng, c, s0, s1):
                eng.dma_start(qt[c][:, s0 * 512:s1 * 512],
                              qt_in[c * 128:(c + 1) * 128, s0 * 512:s1 * 512])

            kt1_dma(nc.sync, 0, 0, 512)
            kt1_dma(nc.gpsimd, 1, 0, 512)
            qt1_dma(nc.scalar, 0, 0, 1)
            qt1_dma(nc.scalar, 1, 0, 1)
            nc.gpsimd.dma_start(mask[:], m_in[:])
            qt1_dma(nc.scalar, 0, 1, 2)
            qt1_dma(nc.scalar, 1, 1, 2)
            v_dma(nc.gpsimd, 0, 4)
            kt1_dma(nc.sync, 0, 512, 1024)
            qt1_dma(nc.sync, 0, 4, 6)
            qt1_dma(nc.scalar, 0, 2, 4)
            qt1_dma(nc.scalar, 1, 2, 4)
            kt1_dma(nc.gpsimd, 1, 512, 1024)
            v_dma(nc.gpsimd, 4, 6)
            kt1_dma(nc.sync, 0, 1024, 2048)
            qt1_dma(nc.sync, 1, 4, 6)
            qt1_dma(nc.scalar, 0, 6, 8)
            qt1_dma(nc.scalar, 1, 6, 8)
            kt1_dma(nc.gpsimd, 1, 1024, 2048)
            v_dma(nc.gpsimd, 10, 6)

            # Global pair stream: slot p = [diag pair p] + pairs t=0..p-1.
            sched = []
            for p in range(NSLOT):
                lst = [p] + list(range(0, p))
                for i, t in enumerate(lst):
                    sched.append((p, t, i == len(lst) - 1))
            ngl = len(sched)  # 44

            # filler matmuls before the first St of early slots (the ~7us
            # framework prologue delays DMA enough that none are needed).
            fills = {}

            pts = {}
            o_ps = [None]

            def emit_st(g):
                p, t, _ = sched[g]
                diag = t == p
                st = stp.tile([128, 1024], F32, tag="st", name=f"st{p}_{t}")
                for _ in range(fills.get(g, 0)):
                    nc.tensor.matmul(st[0:64, 0:64], lhsT=wu[:], rhs=wu[:],
                                     start=True, stop=True)
                if diag:
                    for c in range(2):  # pos 2p vs qs0..3 -> cols 0:512
                        nc.tensor.matmul(
                            st[:, 0:512],
                            lhsT=kt[c][:, 2 * p * 128:(2 * p + 1) * 128],
                            rhs=qt[c][:, p * 512:p * 512 + 512],
                            start=(c == 0), stop=(c == 1),
                        )
                    for c in range(2):  # pos 2p+1 vs qs2,3 -> cols 512:768
                        nc.tensor.matmul(
                            st[:, 512:768],
                            lhsT=kt[c][:, (2 * p + 1) * 128:(2 * p + 2) * 128],
                            rhs=qt[c][:, p * 512 + 256:p * 512 + 512],
                            start=(c == 0), stop=(c == 1),
                        )
                    nc.vector.tensor_tensor(st[:, 0:256], st[:, 0:256], mask[:],
                                            AluAdd)
                    nc.vector.tensor_tensor(st[:, 512:768], st[:, 512:768],
                                            mask[:], AluAdd)
                    w = 768
                else:
                    for jp in range(2):  # pos 2t+jp -> cols jp*512
                        for c in range(2):
                            nc.tensor.matmul(
                                st[:, jp * 512:(jp + 1) * 512],
                                lhsT=kt[c][:, (2 * t + jp) * 128:(2 * t + jp + 1) * 128],
                                rhs=qt[c][:, p * 512:p * 512 + 512],
                                start=(c == 0), stop=(c == 1),
                            )
                    w = 1024
                pt_t = ptp.tile([128, 1024], BF16, tag="pt", name=f"pt{p}_{t}")
                nc.scalar.activation(pt_t[:, 0:w], st[:, 0:w], Exp, scale=SCALE)
                pts[g] = pt_t

            def emit_pv(g):
                p, t, last = sched[g]
                pt_t = pts.pop(g)
                if t == p:  # diag pair: allocate this slot's O set
                    o_ps[0] = [
                        opp.tile([128, VW], F32, tag="o", name=f"o{p}_{qs}")
                        for qs in range(4)
                    ]
                    # pos 2p+1 probs at cols 512:768 -> qs2,3
                    for i, qs in enumerate((2, 3)):
                        nc.tensor.matmul(
                            o_ps[0][qs][:],
                            lhsT=pt_t[:, 512 + i * 128:640 + i * 128],
                            rhs=vsb[:, (2 * p + 1) * VW:(2 * p + 2) * VW],
                            start=True, stop=False,
                        )
                    # pos 2p probs at cols 0:512 -> qs0..3
                    for qs in range(4):
                        nc.tensor.matmul(
                            o_ps[0][qs][:],
                            lhsT=pt_t[:, qs * 128:(qs + 1) * 128],
                            rhs=vsb[:, 2 * p * VW:(2 * p + 1) * VW],
                            start=(qs < 2), stop=last,
                        )
                else:
                    for jp in range(2):
                        for qs in range(4):
                            nc.tensor.matmul(
                                o_ps[0][qs][:],
                                lhsT=pt_t[:, jp * 512 + qs * 128:jp * 512 + (qs + 1) * 128],
                                rhs=vsb[:, (2 * t + jp) * VW:(2 * t + jp + 1) * VW],
                                start=False, stop=(last and jp == 1),
                            )
                if last:
                    ob = osbp.tile([128, 4 * VW], F32, tag="ob", name=f"ob{p}")
                    if p == NSLOT - 1:
                        # tail: drain each quarter as soon as it is copied
                        for qs in range(4):
                            nc.vector.tensor_copy(
                                ob[:, qs * VW:(qs + 1) * VW], o_ps[0][qs][:]
                            )
                            nc.sync.dma_start(
                                o_out[p * 512 + qs * 128:p * 512 + (qs + 1) * 128, :],
                                ob[:, qs * VW:(qs + 1) * VW],
                            )
                    else:
                        # qs2/qs3 first: the next slot's diag PV targets them
                        for qs in (2, 3, 0, 1):
                            nc.vector.tensor_copy(
                                ob[:, qs * VW:(qs + 1) * VW], o_ps[0][qs][:]
                            )
                        nc.sync.dma_start(
                            o_out[p * 512:(p + 1) * 512, :].rearrange(
                                "(qs pp) v -> pp qs v", pp=128
                            ),
                            ob[:].rearrange("p (qs v) -> p qs v", qs=4),
                        )

            for g in range(ngl):
                emit_st(g)
                if g >= 2:
                    emit_pv(g - 2)
            emit_pv(ngl - 2)
            emit_pv(ngl - 1)
    nc.compile()
    return nc


def _get_nc():
    if "nc" not in _CACHE:
        _CACHE["nc"] = _build_nc()
    return _CACHE["nc"]


def _make_mask(h):
    """Additive diag mask [128, 256] fp32 = [E|O] for core-half h.

    E ("even" q sub-block vs its diagonal k-block): partitions 0..63 hold
    rows 64h..64h+63 of the diagonal block (strict lower-triangular),
    partitions 64..127 hold the next block up (fully blocked).
    O ("odd"): partitions 0..63 fully allowed, 64..127 strict diagonal.
    """
    part = np.arange(64)[:, None]
    x = np.arange(128)[None, :]
    strict = np.where(x > 64 * h + part, 0.0, NEG).astype(np.float32)
    e = np.concatenate([strict, np.full((64, 128), NEG, np.float32)], axis=0)
    o = np.concatenate([np.zeros((64, 128), np.float32), strict], axis=0)
    return np.concatenate([e, o], axis=1)


def _pack_rows(x, h):
    """Select rows 64h..64h+63 of every 128-row block: [4096, C]->[2048, C]."""
    return x.reshape(32, 128, -1)[:, 64 * h:64 * h + 64].reshape(2048, -1)


def _run(query, key, value, trace=False, trace_cores=None):
    from concourse.bass_utils import run_bass_kernel_spmd

    query = np.ascontiguousarray(np.asarray(query, dtype=np.float32)).reshape(B, S, C)
    key = np.ascontiguousarray(np.asarray(key, dtype=np.float32)).reshape(B, S, C)
    value = np.ascontiguousarray(np.asarray(value, dtype=np.float32)).reshape(B, S, C)

    masks = [_make_mask(h) for h in range(2)]
    in_maps = []
    for core in range(8):
        b, h = core // 2, core % 2
        v_sel = _pack_rows(value[b], h)
        v257 = np.zeros((NPOS * 128, VW), np.float32)
        v257[:, :C] = v_sel
        v257[:, C] = 1.0
        in_maps.append(
            {
                "qt_in": np.ascontiguousarray(query[b].T).astype(bfloat16),
                "kt_in": np.ascontiguousarray(_pack_rows(key[b], h).T).astype(bfloat16),
                "v_in": v257.astype(bfloat16),
                "m_in": masks[h],
            }
        )

    nc = _get_nc()
    res = run_bass_kernel_spmd(
        nc,
        in_maps,
        list(range(8)),
        trace=trace,
        trace_cores=trace_cores,
    )

    out = np.empty((B, S, C), np.float32)
    for b in range(B):
        o0 = res.results[2 * b]["o_out"].astype(np.float64)
        o1 = res.results[2 * b + 1]["o_out"].astype(np.float64)
        num = o0[:, :C] + o1[:, :C]
        den = o0[:, C] + o1[:, C]
        den = np.where(den == 0.0, 1.0, den)
        out[b] = (num / den[:, None]).astype(np.float32)
    return out.reshape(B, 64, 64, C), res


def kernel(query, key, value):
    out, _ = _run(query, key, value, trace=False)
    return out


# revision 17
# speedup vs baseline: 1.1921x; 1.1921x over previous
"""Trainium2 Bass kernel for PixelSNAIL-style strict-causal attention.

Problem: query/key/value [B=4, H=64, W=64, C=256] fp32.
  S = 4096 tokens per batch; scores = (Q K^T)/16 with strict causal mask
  (position i attends to j < i); out = softmax(scores) @ V (row 0 -> 0).

Strategy (8 NeuronCores, v3):
  - 2 cores per batch, context-parallel over k: core h owns rows
    64h..64h+63 of EVERY 128-row k-block (row-half split). Both cores run
    the IDENTICAL program (SPMD) and their partial numerator/denominator
    outputs are summed on host. The row-half split (vs block parity) makes
    the diagonal-block mask pattern identical across slots AND cores, so a
    single [128, 512] additive-mask input covers everything and the
    per-slot trimming is h-independent.
  - All matmul operands bf16 (fp32 PSUM accumulation). Measured L2 rel
    err ~3e-3 (gate 2e-2). PE streams at ~2.0 GHz under the P0 power
    limit, so the matmul cycle count (~140k/core) is the real floor.
  - No max-subtraction in softmax (scores ~ N(0,1), exp safe in fp32).
  - V gets a ones-column (row 257) so the softmax denominator accumulates
    in PSUM alongside the numerator.
  - q-slots (512 rows) processed ASCENDING as one globally software-
    pipelined stream of k-position PAIRS: St(g+2) is issued before PV(g),
    so each pair's exp (ACT) hides under other pairs' matmuls and the PE
    never drains (keeps the HAM clock gate open). Slot-local order puts
    the diagonal (masked) pair first; PSUM->SBUF output copies drain
    qs2/qs3 first because the next slot's first PV MMs target them.
  - St is pair-granular [128, 1024] (2 PSUM banks) with ONE activation
    per pair: the ~460ns fixed ACT overhead is paid 44x, not 72x.

Layout per core (b = core//2, h = core%2):
  qt_in [256, 4096] bf16 = Q[b]^T
  kt_in [256, 2048] bf16 = row-half-packed K[b]^T (local col 64*blk + r,
                            r in 0..63 <-> global row 128*blk + 64h + r)
  v_in  [2048, 257] bf16 = row-half-packed V[b] ++ ones column
  m_in  [128, 256] fp32  = additive diag mask [E|O]
  o_out [4096, 257] fp32 = partial (numerator ++ denominator)

St pair layouts (matmul PSUM writes must not cross a bank boundary):
  Diagonal pair (slot p, k-positions 2p and 2p+1), width 768:
    cols 0:512   = pos 2p   vs q sub-blocks 0..3 (qs0,1 masked by m)
    cols 512:768 = pos 2p+1 vs q sub-blocks 2,3  (masked by m)
  Non-diag pair (t < p), width 1024:
    cols 0:512 = pos 2t, cols 512:1024 = pos 2t+1, no mask.
"""

import numpy as np
from ml_dtypes import bfloat16

B = 4
S = 4096          # 64*64 tokens per batch
C = 256
NPOS = 16         # 128-row local k positions per core
NSLOT = 8         # q slots of 512 rows
VW = 257          # V width incl. denominator ones-column
SCALE = 1.0 / 16.0
NEG = -1.0e30

_CACHE = {}


def _build_nc():
    import concourse.bacc as bacc
    import concourse.mybir as mybir
    import concourse.tile as tile

    F32 = mybir.dt.float32
    BF16 = mybir.dt.bfloat16
    AluAdd = mybir.AluOpType.add
    Exp = mybir.ActivationFunctionType.Exp

    nc = bacc.Bacc("TRN2", target_bir_lowering=False, debug=False, num_devices=8)
    qt_in = nc.dram_tensor("qt_in", [C, S], BF16, kind="ExternalInput").ap()
    kt_in = nc.dram_tensor("kt_in", [C, NPOS * 128], BF16, kind="ExternalInput").ap()
    v_in = nc.dram_tensor("v_in", [NPOS * 128, VW], BF16, kind="ExternalInput").ap()
    m_in = nc.dram_tensor("m_in", [128, 256], F32, kind="ExternalInput").ap()
    o_out = nc.dram_tensor("o_out", [S, VW], F32, kind="ExternalOutput").ap()

    with tile.TileContext(nc) as tc:
        with (
            tc.tile_pool(name="const", bufs=1) as const,
            tc.tile_pool(name="pt", bufs=3) as ptp,
            tc.tile_pool(name="osb", bufs=3) as osbp,
            tc.tile_pool(name="st", bufs=2, space="PSUM") as stp,
            tc.tile_pool(name="op", bufs=4, space="PSUM") as opp,
        ):
            # PE warmup: tiny matmuls issued during the DMA preamble so the
            # HAM clock gate opens before real work.
            wu = const.tile([128, 64], BF16, tag="wu")
            nc.gpsimd.memset(wu[:], 0.0)
            wu_ps = stp.tile([128, 1024], F32, tag="st", name="wu_ps")
            for _ in range(48):
                nc.tensor.matmul(wu_ps[0:64, 0:64], lhsT=wu[:], rhs=wu[:],
                                 start=True, stop=True)

            qt = [const.tile([128, S], BF16, tag=f"qt{c}", name=f"qt{c}")
                  for c in range(2)]
            kt = [const.tile([128, NPOS * 128], BF16, tag=f"kt{c}", name=f"kt{c}")
                  for c in range(2)]
            vsb = const.tile([128, NPOS * VW], BF16, tag="v")
            mask = const.tile([128, 256], F32, tag="m")

            def kt_dma(eng, c0, c1):
                for c in range(2):
                    eng.dma_start(kt[c][:, c0:c1], kt_in[c * 128:(c + 1) * 128, c0:c1])

            def qt_dma(eng, s0, s1):
                for c in range(2):
                    eng.dma_start(
                        qt[c][:, s0 * 512:s1 * 512],
                        qt_in[c * 128:(c + 1) * 128, s0 * 512:s1 * 512],
                    )

            def v_dma(eng, pos, npos):
                eng.dma_start(
                    vsb[:, pos * VW:(pos + npos) * VW].rearrange(
                        "p (t v) -> p t v", t=npos
                    ),
                    v_in[pos * 128:(pos + npos) * 128, :].rearrange(
                        "(t p) v -> p t v", p=128
                    ),
                )

            # Pre-load the Exp activation table (one-time ~1.3us) on the ACT
            # queue during the preamble, so slot 0's first real exp does not
            # pay it. The probe input must be fp32 like the real exps (the
            # table is input-dtype specific).
            wu32 = const.tile([128, 1], F32, tag="wu32")
            nc.gpsimd.memset(wu32[:], 0.0)
            wu_pt = const.tile([128, 1], BF16, tag="wupt")
            nc.scalar.activation(wu_pt[:], wu32[:], Exp, scale=SCALE)

            # Input DMAs on two parallel queues (sync + gpsimd) in
            # consumption order; outputs appended to the sync queue later.
            kt_dma(nc.sync, 0, 256)
            qt_dma(nc.gpsimd, 0, 1)
            nc.gpsimd.dma_start(mask[:], m_in[:])
            kt_dma(nc.sync, 256, 512)
            v_dma(nc.gpsimd, 0, 2)
            qt_dma(nc.gpsimd, 1, 2)
            kt_dma(nc.sync, 512, 1024)
            v_dma(nc.gpsimd, 2, 6)
            qt_dma(nc.gpsimd, 2, 3)
            qt_dma(nc.sync, 4, 5)
            kt_dma(nc.sync, 1024, 2048)
            qt_dma(nc.gpsimd, 3, 4)
            v_dma(nc.gpsimd, 8, 8)
            qt_dma(nc.sync, 5, 8)

            # Global pair stream: slot p = [diag pair p] + pairs t=0..p-1.
            sched = []
            for p in range(NSLOT):
                lst = [p] + list(range(0, p))
                for i, t in enumerate(lst):
                    sched.append((p, t, i == len(lst) - 1))
            ngl = len(sched)  # 44

            # filler matmuls before the first St of early slots (the ~7us
            # framework prologue delays DMA enough that none are needed).
            fills = {}

            pts = {}
            o_ps = [None]

            def emit_st(g):
                p, t, _ = sched[g]
                diag = t == p
                st = stp.tile([128, 1024], F32, tag="st", name=f"st{p}_{t}")
                for _ in range(fills.get(g, 0)):
                    nc.tensor.matmul(st[0:64, 0:64], lhsT=wu[:], rhs=wu[:],
                                     start=True, stop=True)
                if diag:
                    for c in range(2):  # pos 2p vs qs0..3 -> cols 0:512
                        nc.tensor.matmul(
                            st[:, 0:512],
                            lhsT=kt[c][:, 2 * p * 128:(2 * p + 1) * 128],
                            rhs=qt[c][:, p * 512:p * 512 + 512],
                            start=(c == 0), stop=(c == 1),
                        )
                    for c in range(2):  # pos 2p+1 vs qs2,3 -> cols 512:768
                        nc.tensor.matmul(
                            st[:, 512:768],
                            lhsT=kt[c][:, (2 * p + 1) * 128:(2 * p + 2) * 128],
                            rhs=qt[c][:, p * 512 + 256:p * 512 + 512],
                            start=(c == 0), stop=(c == 1),
                        )
                    nc.vector.tensor_tensor(st[:, 0:256], st[:, 0:256], mask[:],
                                            AluAdd)
                    nc.vector.tensor_tensor(st[:, 512:768], st[:, 512:768],
                                            mask[:], AluAdd)
                    w = 768
                else:
                    for jp in range(2):  # pos 2t+jp -> cols jp*512
                        for c in range(2):
                            nc.tensor.matmul(
                                st[:, jp * 512:(jp + 1) * 512],
                                lhsT=kt[c][:, (2 * t + jp) * 128:(2 * t + jp + 1) * 128],
                                rhs=qt[c][:, p * 512:p * 512 + 512],
                                start=(c == 0), stop=(c == 1),
                            )
                    w = 1024
                pt_t = ptp.tile([128, 1024], BF16, tag="pt", name=f"pt{p}_{t}")
                nc.scalar.activation(pt_t[:, 0:w], st[:, 0:w], Exp, scale=SCALE)
                pts[g] = pt_t

            def emit_pv(g):
                p, t, last = sched[g]
                pt_t = pts.pop(g)
                if t == p:  # diag pair: allocate this slot's O set
                    o_ps[0] = [
                        opp.tile([128, VW], F32, tag="o", name=f"o{p}_{qs}")
                        for qs in range(4)
                    ]
                    # pos 2p+1 probs at cols 512:768 -> qs2,3
                    for i, qs in enumerate((2, 3)):
                        nc.tensor.matmul(
                            o_ps[0][qs][:],
                            lhsT=pt_t[:, 512 + i * 128:640 + i * 128],
                            rhs=vsb[:, (2 * p + 1) * VW:(2 * p + 2) * VW],
                            start=True, stop=False,
                        )
                    # pos 2p probs at cols 0:512 -> qs0..3
                    for qs in range(4):
                        nc.tensor.matmul(
                            o_ps[0][qs][:],
                            lhsT=pt_t[:, qs * 128:(qs + 1) * 128],
                            rhs=vsb[:, 2 * p * VW:(2 * p + 1) * VW],
                            start=(qs < 2), stop=last,
                        )
                else:
                    for jp in range(2):
                        for qs in range(4):
                            nc.tensor.matmul(
                                o_ps[0][qs][:],
                                lhsT=pt_t[:, jp * 512 + qs * 128:jp * 512 + (qs + 1) * 128],
                                rhs=vsb[:, (2 * t + jp) * VW:(2 * t + jp + 1) * VW],
                                start=False, stop=(last and jp == 1),
                            )
                if last:
                    ob = osbp.tile([128, 4 * VW], F32, tag="ob", name=f"ob{p}")
                    if p == NSLOT - 1:
                        # tail: drain each quarter as soon as it is copied
                        for qs in range(4):
                            nc.vector.tensor_copy(
                                ob[:, qs * VW:(qs + 1) * VW], o_ps[0][qs][:]
                            )
                            nc.sync.dma_start(
                                o_out[p * 512 + qs * 128:p * 512 + (qs + 1) * 128, :],
                                ob[:, qs * VW:(qs + 1) * VW],
                            )
                    else:
                        # qs2/qs3 first: the next slot's diag PV targets them
                        for qs in (2, 3, 0, 1):
                            nc.vector.tensor_copy(
                                ob[:, qs * VW:(qs + 1) * VW], o_ps[0][qs][:]
                            )
                        nc.sync.dma_start(
                            o_out[p * 512:(p + 1) * 512, :].rearrange(
                                "(qs pp) v -> pp qs v", pp=128
                            ),
                            ob[:].rearrange("p (qs v) -> p qs v", qs=4),
                        )

            for g in range(ngl):
                emit_st(g)
                if g >= 2:
                    emit_pv(g - 2)
            emit_pv(ngl - 2)
            emit_pv(ngl - 1)
    nc.compile()
    return nc


def _get_nc():
    if "nc" not in _CACHE:
        _CACHE["nc"] = _build_nc()
    return _CACHE["nc"]


def _make_mask(h):
    """Additive diag mask [128, 256] fp32 = [E|O] for core-half h.

    E ("even" q sub-block vs its diagonal k-block): partitions 0..63 hold
    rows 64h..64h+63 of the diagonal block (strict lower-triangular),
    partitions 64..127 hold the next block up (fully blocked).
    O ("odd"): partitions 0..63 fully allowed, 64..127 strict diagonal.
    """
    part = np.arange(64)[:, None]
    x = np.arange(128)[None, :]
    strict = np.where(x > 64 * h + part, 0.0, NEG).astype(np.float32)
    e = np.concatenate([strict, np.full((64, 128), NEG, np.float32)], axis=0)
    o = np.concatenate([np.zeros((64, 128), np.float32), strict], axis=0)
    return np.concatenate([e, o], axis=1)


def _pack_rows(x, h):
    """Select rows 64h..64h+63 of every 128-row block: [4096, C]->[2048, C]."""
    return x.reshape(32, 128, -1)[:, 64 * h:64 * h + 64].reshape(2048, -1)


def _run(query, key, value, trace=False, trace_cores=None):
    from concourse.bass_utils import run_bass_kernel_spmd

    query = np.ascontiguousarray(np.asarray(query, dtype=np.float32)).reshape(B, S, C)
    key = np.ascontiguousarray(np.asarray(key, dtype=np.float32)).reshape(B, S, C)
    value = np.ascontiguousarray(np.asarray(value, dtype=np.float32)).reshape(B, S, C)

    masks = [_make_mask(h) for h in range(2)]
    in_maps = []
    for core in range(8):
        b, h = core // 2, core % 2
        v_sel = _pack_rows(value[b], h)
        v257 = np.zeros((NPOS * 128, VW), np.float32)
        v257[:, :C] = v_sel
        v257[:, C] = 1.0
        in_maps.append(
            {
                "qt_in": np.ascontiguousarray(query[b].T).astype(bfloat16),
                "kt_in": np.ascontiguousarray(_pack_rows(key[b], h).T).astype(bfloat16),
                "v_in": v257.astype(bfloat16),
                "m_in": masks[h],
            }
        )

    nc = _get_nc()
    res = run_bass_kernel_spmd(
        nc,
        in_maps,
        list(range(8)),
        trace=trace,
        trace_cores=trace_cores,
    )

    out = np.empty((B, S, C), np.float32)
    for b in range(B):
        o0 = res.results[2 * b]["o_out"].astype(np.float64)
        o1 = res.results[2 * b + 1]["o_out"].astype(np.float64)
        num = o0[:, :C] + o1[:, :C]
        den = o0[:, C] + o1[:, C]
        den = np.where(den == 0.0, 1.0, den)
        out[b] = (num / den[:, None]).astype(np.float32)
    return out.reshape(B, 64, 64, C), res


def kernel(query, key, value):
    out, _ = _run(query, key, value, trace=False)
    return out
